# revision 33
# baseline (speedup 1.0000x reference)
"""nn_Attention Trainium2 Bass kernel.

Full attention forward: x->(q,k,v) with l2-normalized weights, per-head-dim
l2 norm + learned qk scale, interleaved RoPE, causal SDPA, output projection
with column-l2-normalized wo.

Sharding: TP=4 over heads (8 heads/core) x DP=2 over batch across 8 cores.
Each core computes a partial [2048, 2048] output for its batch; host sums
the 4 TP partials per batch.

Device layout tricks:
- rope-pair permutation (evens|odds blocks) folded into wq/wk rows host-side,
  so RoPE's rotate-half becomes a contiguous 32-column swap; rope multiplies
  fold the swap into strided tensor_tensor views (no copies).
- qk_scale folded into the cos/sin tables host-side.
- proj phase and attn phase each self-dense (phase-scoped PSUM pools);
  yproj(i) matmuls fill attn(i+1)'s exp-latency stalls (keeps PE HAM-warm).
- per su-block all 4 PE transposes land in one PSUM bank, extracted by a
  single strided DVE copy (4x fewer copies).
- per-head l2 norm: 1/sqrt(ssq) computed as exp(-0.5*ln(ssq)) so exp/ln/
  square/copy all live in one ACT table set (no table thrashing).
- transposed softmax: logitsT [sj, si] blocks; logits are bounded
  (|logit| <= max(qk_scale)^2), so exp without max subtraction is safe;
  causal mask applied as a 0/1 multiply on diagonal blocks; PV consumes
  attnT directly; softmax denominators from a ones-column matmul alongside
  PV; 1/denom via reciprocal_approx_fast + K=2 broadcast matmul per pair.
- QK matmuls for a head pair issued adjacently at base partitions 0/64 so
  the two K=64 matmuls run in separate PE row-groups concurrently.
- attention output overwrites qT storage (disjoint column ranges already
  consumed), saving 4MB of SBUF.
"""
import sys
import os
import math
from contextlib import ExitStack

sys.path.insert(0, "/opt/trn_rl_repo")

import numpy as np
import ml_dtypes

BF16 = ml_dtypes.bfloat16

B, S, DIM = 2, 2048, 2048
HEADS, DH = 32, 64
THETA = 10000.0
NCORES = 8
TP = 4             # head-parallel ways
HPC = HEADS // TP  # heads per core = 8
E = HPC * DH       # per-core qkv width = 512
ET = E // 128      # e-tiles per core = 4
DT = DIM // 128    # contraction d-tiles = 16
SB = S // 512      # 512-wide seq blocks = 4
SS = S // 128      # 128-wide seq blocks = 16

_CACHE = {}


def _l2n(w, axis):
    n = np.sqrt((w.astype(np.float64) ** 2).sum(axis=axis, keepdims=True))
    n = np.maximum(n, 1e-12)
    return (w / n).astype(np.float32)


def _build_program():
    import concourse.bass as bass
    from concourse import bacc
    import concourse.mybir as mybir
    import concourse.tile as tile
    from concourse.masks import make_identity

    f32 = mybir.dt.float32
    bf16 = mybir.dt.bfloat16
    AF = mybir.ActivationFunctionType

    nc = bacc.Bacc("TRN2", target_bir_lowering=False)

    xT = nc.dram_tensor("xT", [DIM, S], bf16, kind="ExternalInput")
    wqT = nc.dram_tensor("wqT", [DIM, E], bf16, kind="ExternalInput")
    wkT = nc.dram_tensor("wkT", [DIM, E], bf16, kind="ExternalInput")
    wvT = nc.dram_tensor("wvT", [DIM, E], bf16, kind="ExternalInput")
    woT = nc.dram_tensor("woT", [E, DIM], bf16, kind="ExternalInput")
    cosd = nc.dram_tensor("cosd", [128, SS * DH], bf16, kind="ExternalInput")
    sind = nc.dram_tensor("sind", [128, SS * DH], bf16, kind="ExternalInput")
    maskd = nc.dram_tensor("maskd", [128, 4 * 512], bf16, kind="ExternalInput")
    Y = nc.dram_tensor("Y", [S, DIM], bf16, kind="ExternalOutput")

    with tile.TileContext(nc) as tc, ExitStack() as ctx:
        const = ctx.enter_context(tc.tile_pool(name="const", bufs=1))
        wpool = ctx.enter_context(tc.tile_pool(name="wpool", bufs=4))
        xpool = ctx.enter_context(tc.tile_pool(name="xpool", bufs=1))
        qkv = ctx.enter_context(tc.tile_pool(name="qkv", bufs=1))
        work = ctx.enter_context(tc.tile_pool(name="work", bufs=1))
        attnp = ctx.enter_context(tc.tile_pool(name="attnp", bufs=3))
        ypool = ctx.enter_context(tc.tile_pool(name="ypool", bufs=2))
        pools = {}

        # load order tracks first-use: wq, x(st0), wk, x(st1), wv, ...
        wq_sb = wpool.tile([128, DT, E], bf16, tag="w")
        wk_sb = wpool.tile([128, DT, E], bf16, tag="w")
        wv_sb = wpool.tile([128, DT, E], bf16, tag="w")
        wo_sb = wpool.tile([128, ET, DIM], bf16, tag="w")
        cos_sb = const.tile([128, SS, DH], bf16)
        sin_sb = const.tile([128, SS, DH], bf16)
        mask_sb = const.tile([128, 4, 512], bf16)
        xfull = xpool.tile([128, DT, S], bf16, tag="xf")
        nc.sync.dma_start(wq_sb, wqT.rearrange("(t p) e -> p t e", p=128))

        def load_x(st0):
            for dt0 in range(DT):
                nc.sync.dma_start(
                    xfull[:, dt0, st0 * 512:(st0 + 1) * 512],
                    xT[dt0 * 128:(dt0 + 1) * 128, st0 * 512:(st0 + 1) * 512])

        load_x(0)
        nc.sync.dma_start(wk_sb, wkT.rearrange("(t p) e -> p t e", p=128))
        nc.sync.dma_start(wv_sb, wvT.rearrange("(t p) e -> p t e", p=128))
        load_x(1)
        load_x(2)
        nc.sync.dma_start(cos_sb, cosd.rearrange("p (b d) -> p b d", d=DH))
        nc.sync.dma_start(sin_sb, sind.rearrange("p (b d) -> p b d", d=DH))
        nc.sync.dma_start(mask_sb, maskd.rearrange("p (r n) -> p r n", n=512))
        load_x(3)
        nc.sync.dma_start(wo_sb, woT.rearrange("(t p) e -> p t e", p=128))
        # bc matmul lhsT: row 0 -> cols 0:64, row 32 -> cols 64:128
        onemap = const.tile([33, 128], bf16)
        nc.vector.memset(onemap, 0.0)
        nc.vector.memset(onemap[0:1, 0:64], 1.0)
        nc.vector.memset(onemap[32:33, 64:128], 1.0)
        den33 = const.tile([33, 512], f32)   # rows 0/32: 1/denominator
        nc.vector.memset(den33, 1.0)
        den33b = const.tile([33, 512], bf16)
        nc.vector.memset(den33b, 0.0)
        ident = const.tile([128, 128], bf16)
        make_identity(nc, ident)

        # persistent activations; qTall doubles as attention-output storage.
        # layout [e-within-et, et, s]
        qTall = qkv.tile([128, ET, S], bf16, tag="qT", name="qTall")
        kTall = qkv.tile([128, ET, S], bf16, tag="kT", name="kTall")
        v_sb = qkv.tile([128, SS, HPC, 65], bf16, tag="v")
        nc.vector.memset(v_sb[:, :, :, 64:65], 1.0)

        def proj_chain(ps, dstT, st, su):
            """psum [si,e] natural -> per-head l2norm, rope, bf16,
            -> PE transpose into dstT[:, :, sblk*128:+128].
            First op casts PSUM->SBUF so the PSUM bank frees fast."""
            sblk = st * 4 + su
            sq = work.tile([128, E], bf16, tag="sq", bufs=2)
            nc.scalar.square(sq, ps)
            ssq = work.tile([128, HPC], f32, tag="ssq", bufs=2)
            nc.vector.tensor_reduce(
                ssq, sq.rearrange("p (h d) -> p h d", d=DH),
                axis=mybir.AxisListType.X, op=mybir.AluOpType.add)
            # inv = ssq^-0.5 = exp(-0.5*ln(ssq)); keeps one ACT table set
            lns = work.tile([128, HPC], f32, tag="lns", bufs=2)
            nc.scalar.activation(lns, ssq, AF.Ln)
            inv = work.tile([128, HPC], f32, tag="inv", bufs=2)
            nc.scalar.activation(inv, lns, AF.Exp, scale=-0.5)
            qn = work.tile([128, HPC, DH], bf16, tag="qn", bufs=2)
            nc.vector.tensor_mul(
                qn, ps.rearrange("p (h d) -> p h d", d=DH),
                inv.unsqueeze(2).broadcast_to([128, HPC, DH]))
            cosb = cos_sb[:, sblk:sblk + 1, :].broadcast_to([128, HPC, DH])
            sinb = sin_sb[:, sblk:sblk + 1, :].broadcast_to([128, HPC, DH])
            qn4 = qn.rearrange("p h (t u) -> p h t u", u=32)
            rot = work.tile([128, HPC, 2, 32], bf16, tag="rot", bufs=2)
            sinb4 = sinb.rearrange("p h (t u) -> p h t u", u=32)
            nc.vector.tensor_mul(rot[:, :, 0:1, :], qn4[:, :, 1:2, :],
                                 sinb4[:, :, 0:1, :])
            nc.vector.tensor_mul(rot[:, :, 1:2, :], qn4[:, :, 0:1, :],
                                 sinb4[:, :, 1:2, :])
            nc.vector.tensor_mul(qn, qn, cosb)  # in-place: qn *= cos
            qo = work.tile([128, E], bf16, tag="qo", bufs=2)
            nc.vector.tensor_add(
                qo, qn.rearrange("p h d -> p (h d)"),
                rot.rearrange("p h t u -> p (h t u)"))
            # 4 PE transposes into one PSUM bank, one strided copy out
            trp = pools["trpp"].tile([128, ET, 128], bf16, tag="trp",
                                     name=f"trp{sblk}")
            for et in range(ET):
                nc.tensor.transpose(trp[:, et, :],
                                    qo[:, et * 128:(et + 1) * 128], ident)
            nc.vector.tensor_copy(
                dstT[:, :, sblk * 128:(sblk + 1) * 128], trp)

        def proj_wave(w_sb, dstT, st, su):
            """project one su-128 block of si-512 block st for one tensor."""
            ps = pools["util"].tile([128, E], f32, tag="u512",
                                    name=f"pp{st}_{su}")
            for dt in range(DT):
                nc.tensor.matmul(
                    ps,
                    xfull[:, dt, st * 512 + su * 128:
                          st * 512 + (su + 1) * 128],
                    w_sb[:, dt, :],
                    start=(dt == 0), stop=(dt == DT - 1))
            if dstT is None:
                nc.vector.tensor_copy(
                    v_sb[:, st * 4 + su, :, 0:64],
                    ps.rearrange("p (h d) -> p h d", d=DH))
            else:
                proj_chain(ps, dstT, st, su)

        def attn_pair(et, i):
            """head pair (2*et, 2*et+1), si-512 block i: QK/exp/mask/PV and
            softmax division; writes attn output into qTall block i."""
            hA, hB = 2 * et, 2 * et + 1
            nsj = 4 * i + 4
            pvA = pools["pvp"].tile([128, 512], f32, tag="pv",
                                    name=f"pvA{et}_{i}")
            pvB = pools["pvp"].tile([128, 512], f32, tag="pv",
                                    name=f"pvB{et}_{i}")
            exs = {}

            def emit_qk(sjb):
                lg = pools["lgp"].tile([128, 2, 512], f32, tag="lg",
                                       name=f"lg{et}_{i}_{sjb}")
                c0 = sjb * 128
                nc.tensor.matmul(
                    lg[:, 0, :],
                    kTall[0:64, et, c0:c0 + 128],
                    qTall[0:64, et, i * 512:(i + 1) * 512],
                    start=True, stop=True)
                nc.tensor.matmul(
                    lg[:, 1, :],
                    kTall[64:128, et, c0:c0 + 128],
                    qTall[64:128, et, i * 512:(i + 1) * 512],
                    start=True, stop=True)
                ex = attnp.tile([128, 2, 512], bf16, tag="ex", bufs=3)
                nc.scalar.activation(ex, lg, AF.Exp)
                r = sjb - 4 * i
                if r >= 0:
                    nc.vector.tensor_mul(
                        ex, ex,
                        mask_sb[:, r:r + 1, :].broadcast_to([128, 2, 512]))
                exs[sjb] = ex

            emit_qk(0)
            for sjb in range(nsj):
                if sjb + 1 < nsj:
                    emit_qk(sjb + 1)
                ex = exs.pop(sjb)
                nc.tensor.matmul(
                    pvA[0:65, :], v_sb[:, sjb, hA, :], ex[:, 0, :],
                    start=(sjb == 0), stop=(sjb == nsj - 1))
                nc.tensor.matmul(
                    pvB[0:65, :], v_sb[:, sjb, hB, :], ex[:, 1, :],
                    start=(sjb == 0), stop=(sjb == nsj - 1))

            # softmax denominators -> broadcast 1/den -> scale outputs
            nc.vector.tensor_copy(den33[0:1, :], pvA[64:65, :])
            nc.vector.tensor_copy(den33[32:33, :], pvB[64:65, :])
            # in-place 1/x over rows 0..32 (rows 1-31 unused; base-partition-0
            # form because custom-DVE dst at base 32 miswrites on HW)
            nc.vector.reciprocal_approx_fast(den33, den33)
            nc.vector.tensor_copy(den33b[0:1, :], den33[0:1, :])
            nc.vector.tensor_copy(den33b[32:33, :], den33[32:33, :])
            bc = pools["lgp"].tile([128, 2, 512], f32, tag="lg",
                                   name=f"bc{et}_{i}")
            nc.tensor.matmul(bc[:, 0, :], onemap, den33b, start=True,
                             stop=True)
            bcs = attnp.tile([128, 512], bf16, tag="bcs", bufs=2)
            nc.vector.tensor_copy(bcs, bc[:, 0, :])
            nc.vector.tensor_mul(
                qTall[0:64, et, i * 512:(i + 1) * 512],
                pvA[0:64, :], bcs[0:64, :])
            nc.vector.tensor_mul(
                qTall[64:128, et, i * 512:(i + 1) * 512],
                pvB[0:64, :], bcs[64:128, :])

        def yproj_group(ib, nd):
            """one nd-512 chunk of si-128 block ib -> Y[ib*128:+128, nd*512:]."""
            ps = pools["util2"].tile([128, 512], f32, tag="y512",
                                     name=f"yp{ib}_{nd}")
            for ket in range(ET):
                nc.tensor.matmul(
                    ps,
                    qTall[:, ket, ib * 128:(ib + 1) * 128],
                    wo_sb[:, ket, nd * 512:(nd + 1) * 512],
                    start=(ket == 0), stop=(ket == ET - 1))
            ys = ypool.tile([128, 512], bf16, tag="y")
            nc.vector.tensor_copy(ys, ps)
            nc.sync.dma_start(
                Y[ib * 128:(ib + 1) * 128, nd * 512:(nd + 1) * 512], ys)

        # proj phase (own PSUM pools: 4 wave banks + 4 transpose banks)
        with tc.tile_pool(name="util", bufs=4, space="PSUM") as p_util, \
             tc.tile_pool(name="trpp", bufs=4, space="PSUM") as p_trpp:
            pools["util"] = p_util
            pools["trpp"] = p_trpp
            for st in range(SB):
                for su in range(4):
                    proj_wave(wq_sb, qTall, st, su)
                for su in range(4):
                    proj_wave(wk_sb, kTall, st, su)
                for su in range(4):
                    proj_wave(wv_sb, None, st, su)
        # attn phase: lg 2x2 banks + pv 3 + yproj/bc 1
        with tc.tile_pool(name="lgp", bufs=2, space="PSUM") as p_lgp, \
             tc.tile_pool(name="pvp", bufs=3, space="PSUM") as p_pvp, \
             tc.tile_pool(name="util2", bufs=1, space="PSUM") as p_util2:
            pools["lgp"] = p_lgp
            pools["pvp"] = p_pvp
            pools["util2"] = p_util2
            for i in range(SB):
                for et in range(ET):
                    attn_pair(et, i)
                    if i >= 1:
                        for nd in range(4):
                            yproj_group(4 * (i - 1) + et, nd)
            for ib in range(12, 16):
                for nd in range(4):
                    yproj_group(ib, nd)
    return nc


def _host_prep(x, wq, wk, wv, wo, qk_scale):
    """Returns per-core input dicts."""
    perm = np.concatenate([np.arange(0, DH, 2), np.arange(1, DH, 2)])
    wq_n = _l2n(wq, -1).reshape(HEADS, DH, DIM)[:, perm, :].reshape(HEADS * DH, DIM)
    wk_n = _l2n(wk, -1).reshape(HEADS, DH, DIM)[:, perm, :].reshape(HEADS * DH, DIM)
    wv_n = _l2n(wv, -1)
    wo_n = _l2n(wo, 0)
    sp = qk_scale.astype(np.float64)[perm]

    # rope tables with qk_scale folded in; permuted-block layout
    half = np.arange(0, DH, 2)
    freqs = 1.0 / (THETA ** (half.astype(np.float64) / DH))      # (32,)
    ang = np.arange(S, dtype=np.float64)[:, None] * freqs[None]  # (S, 32)
    cos_h, sin_h = np.cos(ang), np.sin(ang)
    cos_p = np.concatenate([cos_h, cos_h], 1)                    # (S, 64)
    sin_e = np.concatenate([-sin_h, sin_h], 1)
    cos_eff = (cos_p * sp[None, :]).astype(np.float32)
    swap_sp = np.concatenate([sp[32:], sp[:32]])
    sin_eff = (sin_e * swap_sp[None, :]).astype(np.float32)
    # device layout [128, SS*DH]: [p, b*64+c] = tbl[b*128+p, c]
    cosd = np.ascontiguousarray(
        cos_eff.reshape(SS, 128, DH).transpose(1, 0, 2).reshape(128, SS * DH))
    sind = np.ascontiguousarray(
        sin_eff.reshape(SS, 128, DH).transpose(1, 0, 2).reshape(128, SS * DH))

    # causal masks for the 4 diagonal offsets: keep sjl + 128r <= sil
    sjl = np.arange(128)[:, None]
    sil = np.arange(512)[None, :]
    maskd = np.ascontiguousarray(np.concatenate(
        [(sjl + 128 * r <= sil).astype(np.float32) for r in range(4)],
        axis=1))  # [128, 4*512]

    in_maps = []
    for c in range(NCORES):
        b, t = divmod(c, TP)
        e0 = t * E
        in_maps.append({
            "xT": np.ascontiguousarray(x[b].T).astype(BF16),
            "wqT": np.ascontiguousarray(wq_n[e0:e0 + E].T).astype(BF16),
            "wkT": np.ascontiguousarray(wk_n[e0:e0 + E].T).astype(BF16),
            "wvT": np.ascontiguousarray(wv_n[e0:e0 + E].T).astype(BF16),
            "woT": np.ascontiguousarray(wo_n[:, e0:e0 + E].T).astype(BF16),
            "cosd": cosd.astype(BF16), "sind": sind.astype(BF16),
            "maskd": maskd.astype(BF16),
        })
    return in_maps


def _install_profile_hook():
    """antenv.axon_hooks is absent in this image; shim it and register the
    ctypes NTFF hook against /opt/axon/libaxon_pjrt.so (mirrors trn_boot)."""
    import types
    import ctypes
    import contextlib

    try:
        from antenv.axon_hooks import get_axon_ntff_profile_hook  # noqa
        return
    except ImportError:
        pass
    import antenv
    mod = types.ModuleType("antenv.axon_hooks")
    state = {}
    mod.set_axon_ntff_profile_hook = lambda h: state.__setitem__("h", h)
    mod.get_axon_ntff_profile_hook = lambda: state.get("h")
    sys.modules["antenv.axon_hooks"] = mod
    antenv.axon_hooks = mod

    so_path = "/opt/axon/libaxon_pjrt.so"
    lib = ctypes.CDLL(so_path)
    if not hasattr(lib, "axon_start_nrt_profile"):
        return
    lib.axon_start_nrt_profile.argtypes = [
        ctypes.POINTER(ctypes.c_int64), ctypes.c_size_t]
    lib.axon_start_nrt_profile.restype = ctypes.c_int64
    lib.axon_stop_nrt_profile.argtypes = [ctypes.c_char_p]
    lib.axon_stop_nrt_profile.restype = ctypes.c_int64

    @contextlib.contextmanager
    def _hook(output_dir, device_ids):
        import jax
        jax.devices()
        if device_ids:
            ids = (ctypes.c_int64 * len(device_ids))(*device_ids)
            rc = lib.axon_start_nrt_profile(ids, len(device_ids))
        else:
            rc = lib.axon_start_nrt_profile(None, 0)
        if rc != 0:
            raise RuntimeError(f"axon_start_nrt_profile rc={rc}")
        try:
            yield
        finally:
            n = lib.axon_stop_nrt_profile(str(output_dir).encode())
            print(f"profile: {n} file(s) written to {output_dir}",
                  file=sys.stderr)

    mod.set_axon_ntff_profile_hook(_hook)


def kernel(x, wq, wk, wv, wo, qk_scale, _profile=False):
    from concourse.bass_utils import run_bass_kernel_spmd

    if _profile:
        _install_profile_hook()

    if "nc" not in _CACHE:
        nc = _build_program()
        nc.finalize()
        _CACHE["nc"] = nc
    nc = _CACHE["nc"]
    in_maps = _host_prep(np.asarray(x), np.asarray(wq), np.asarray(wk),
                         np.asarray(wv), np.asarray(wo), np.asarray(qk_scale))
    res = run_bass_kernel_spmd(nc, in_maps, core_ids=list(range(NCORES)),
                               trace=_profile)
    outs = res.results
    y = np.empty((B, S, DIM), dtype=np.float32)
    for b in range(B):
        y[b] = sum(np.asarray(outs[b * TP + t]["Y"], dtype=np.float32)
                   for t in range(TP))
    if _profile:
        _CACHE["last_exec_time_ns"] = res.exec_time_ns
        _CACHE["last_profile"] = res.profile_json
    return y


# revision 34
# speedup vs baseline: 1.2083x; 1.2083x over previous
"""nn_Attention Trainium2 Bass kernel.

Full attention forward: x->(q,k,v) with l2-normalized weights, per-head-dim
l2 norm + learned qk scale, interleaved RoPE, causal SDPA, output projection
with column-l2-normalized wo.

Sharding: TP=4 over heads (8 heads/core) x DP=2 over batch across 8 cores.
Each core computes a partial [2048, 2048] output for its batch; host sums
the 4 TP partials per batch.

Device layout tricks:
- rope-pair permutation (evens|odds blocks) folded into wq/wk rows host-side,
  so RoPE's rotate-half becomes a contiguous 32-column swap; rope multiplies
  fold the swap into strided tensor_tensor views (no copies).
- qk_scale folded into the cos/sin tables host-side.
- proj phase and attn phase each self-dense (phase-scoped PSUM pools);
  yproj(i) matmuls fill attn(i+1)'s exp-latency stalls (keeps PE HAM-warm).
- per su-block all 4 PE transposes land in one PSUM bank, extracted by a
  single strided DVE copy (4x fewer copies).
- per-head l2 norm: 1/sqrt(ssq) computed as exp(-0.5*ln(ssq)) so exp/ln/
  square/copy all live in one ACT table set (no table thrashing).
- transposed softmax: logitsT [sj, si] blocks; logits are bounded
  (|logit| <= max(qk_scale)^2), so exp without max subtraction is safe;
  causal mask applied as a 0/1 multiply on diagonal blocks; PV consumes
  attnT directly; softmax denominators from a ones-column matmul alongside
  PV; 1/denom via reciprocal_approx_fast + K=2 broadcast matmul per pair.
- QK matmuls for a head pair issued adjacently at base partitions 0/64 so
  the two K=64 matmuls run in separate PE row-groups concurrently.
- attention output overwrites qT storage (disjoint column ranges already
  consumed), saving 4MB of SBUF.
"""
import sys
import os
import math
from contextlib import ExitStack

sys.path.insert(0, "/opt/trn_rl_repo")

import numpy as np
import ml_dtypes

BF16 = ml_dtypes.bfloat16

B, S, DIM = 2, 2048, 2048
HEADS, DH = 32, 64
THETA = 10000.0
NCORES = 8
TP = 4             # head-parallel ways
HPC = HEADS // TP  # heads per core = 8
E = HPC * DH       # per-core qkv width = 512
ET = E // 128      # e-tiles per core = 4
DT = DIM // 128    # contraction d-tiles = 16
SB = S // 512      # 512-wide seq blocks = 4
SS = S // 128      # 128-wide seq blocks = 16

_CACHE = {}


def _l2n(w, axis):
    n = np.sqrt((w.astype(np.float64) ** 2).sum(axis=axis, keepdims=True))
    n = np.maximum(n, 1e-12)
    return (w / n).astype(np.float32)


def _build_program():
    import concourse.bass as bass
    from concourse import bacc
    import concourse.mybir as mybir
    import concourse.tile as tile
    from concourse.masks import make_identity

    f32 = mybir.dt.float32
    bf16 = mybir.dt.bfloat16
    AF = mybir.ActivationFunctionType

    nc = bacc.Bacc("TRN2", target_bir_lowering=False)

    xT = nc.dram_tensor("xT", [DIM, S], bf16, kind="ExternalInput")
    wqT = nc.dram_tensor("wqT", [DIM, E], bf16, kind="ExternalInput")
    wkT = nc.dram_tensor("wkT", [DIM, E], bf16, kind="ExternalInput")
    wvT = nc.dram_tensor("wvT", [DIM, E], bf16, kind="ExternalInput")
    woT = nc.dram_tensor("woT", [E, DIM], bf16, kind="ExternalInput")
    cosd = nc.dram_tensor("cosd", [128, SS * DH], bf16, kind="ExternalInput")
    sind = nc.dram_tensor("sind", [128, SS * DH], bf16, kind="ExternalInput")
    maskd = nc.dram_tensor("maskd", [128, 4 * 512], bf16, kind="ExternalInput")
    Y = nc.dram_tensor("Y", [S, DIM], bf16, kind="ExternalOutput")

    with tile.TileContext(nc) as tc, ExitStack() as ctx:
        const = ctx.enter_context(tc.tile_pool(name="const", bufs=1))
        wpool = ctx.enter_context(tc.tile_pool(name="wpool", bufs=4))
        xpool = ctx.enter_context(tc.tile_pool(name="xpool", bufs=1))
        qkv = ctx.enter_context(tc.tile_pool(name="qkv", bufs=1))
        work = ctx.enter_context(tc.tile_pool(name="work", bufs=1))
        attnp = ctx.enter_context(tc.tile_pool(name="attnp", bufs=3))
        ypool = ctx.enter_context(tc.tile_pool(name="ypool", bufs=2))
        pools = {}

        # load order tracks first-use: wq, x(st0), wk, x(st1), wv, ...
        wq_sb = wpool.tile([128, DT, E], bf16, tag="w")
        wk_sb = wpool.tile([128, DT, E], bf16, tag="w")
        wv_sb = wpool.tile([128, DT, E], bf16, tag="w")
        wo_sb = wpool.tile([128, ET, DIM], bf16, tag="w")
        cos_sb = const.tile([128, SS, DH], bf16)
        sin_sb = const.tile([128, SS, DH], bf16)
        mask_sb = const.tile([128, 4, 512], bf16)
        xfull = xpool.tile([128, DT, S], bf16, tag="xf")
        nc.sync.dma_start(wq_sb, wqT.rearrange("(t p) e -> p t e", p=128))

        def load_x(st0):
            for dt0 in range(DT):
                nc.sync.dma_start(
                    xfull[:, dt0, st0 * 512:(st0 + 1) * 512],
                    xT[dt0 * 128:(dt0 + 1) * 128, st0 * 512:(st0 + 1) * 512])

        load_x(0)
        nc.sync.dma_start(wk_sb, wkT.rearrange("(t p) e -> p t e", p=128))
        nc.sync.dma_start(wv_sb, wvT.rearrange("(t p) e -> p t e", p=128))
        load_x(1)
        load_x(2)
        nc.sync.dma_start(cos_sb, cosd.rearrange("p (b d) -> p b d", d=DH))
        nc.sync.dma_start(sin_sb, sind.rearrange("p (b d) -> p b d", d=DH))
        nc.sync.dma_start(mask_sb, maskd.rearrange("p (r n) -> p r n", n=512))
        load_x(3)
        nc.sync.dma_start(wo_sb, woT.rearrange("(t p) e -> p t e", p=128))
        # bc matmul lhsT: row 0 -> cols 0:64, row 32 -> cols 64:128
        onemap = const.tile([33, 128], bf16)
        nc.vector.memset(onemap, 0.0)
        nc.vector.memset(onemap[0:1, 0:64], 1.0)
        nc.vector.memset(onemap[32:33, 64:128], 1.0)
        den33 = const.tile([33, 512], f32)   # rows 0/32: 1/denominator
        nc.vector.memset(den33, 1.0)
        den33b = const.tile([33, 512], bf16)
        nc.vector.memset(den33b, 0.0)
        ident = const.tile([128, 128], bf16)
        make_identity(nc, ident)

        # persistent activations; qTall doubles as attention-output storage.
        # layout [e-within-et, et, s]
        qTall = qkv.tile([128, ET, S], bf16, tag="qT", name="qTall")
        kTall = qkv.tile([128, ET, S], bf16, tag="kT", name="kTall")
        v_sb = qkv.tile([128, SS, HPC, 65], bf16, tag="v")
        nc.vector.memset(v_sb[:, :, :, 64:65], 1.0)

        def proj_chain(ps, dstT, st, su):
            """psum [si,e] natural -> per-head l2norm, rope, bf16,
            -> PE transpose into dstT[:, :, sblk*128:+128].
            First op casts PSUM->SBUF so the PSUM bank frees fast."""
            sblk = st * 4 + su
            sq = work.tile([128, E], bf16, tag="sq", bufs=2)
            nc.scalar.square(sq, ps)
            ssq = work.tile([128, HPC], f32, tag="ssq", bufs=2)
            nc.vector.tensor_reduce(
                ssq, sq.rearrange("p (h d) -> p h d", d=DH),
                axis=mybir.AxisListType.X, op=mybir.AluOpType.add)
            # inv = ssq^-0.5 = exp(-0.5*ln(ssq)); keeps one ACT table set
            lns = work.tile([128, HPC], f32, tag="lns", bufs=2)
            nc.scalar.activation(lns, ssq, AF.Ln)
            inv = work.tile([128, HPC], f32, tag="inv", bufs=2)
            nc.scalar.activation(inv, lns, AF.Exp, scale=-0.5)
            qn = work.tile([128, HPC, DH], bf16, tag="qn", bufs=2)
            nc.vector.tensor_mul(
                qn, ps.rearrange("p (h d) -> p h d", d=DH),
                inv.unsqueeze(2).broadcast_to([128, HPC, DH]))
            cosb = cos_sb[:, sblk:sblk + 1, :].broadcast_to([128, HPC, DH])
            sinb = sin_sb[:, sblk:sblk + 1, :].broadcast_to([128, HPC, DH])
            qn4 = qn.rearrange("p h (t u) -> p h t u", u=32)
            rot = work.tile([128, HPC, 2, 32], bf16, tag="rot", bufs=2)
            sinb4 = sinb.rearrange("p h (t u) -> p h t u", u=32)
            nc.vector.tensor_mul(rot[:, :, 0:1, :], qn4[:, :, 1:2, :],
                                 sinb4[:, :, 0:1, :])
            nc.vector.tensor_mul(rot[:, :, 1:2, :], qn4[:, :, 0:1, :],
                                 sinb4[:, :, 1:2, :])
            nc.vector.tensor_mul(qn, qn, cosb)  # in-place: qn *= cos
            qo = work.tile([128, E], bf16, tag="qo", bufs=2)
            nc.vector.tensor_add(
                qo, qn.rearrange("p h d -> p (h d)"),
                rot.rearrange("p h t u -> p (h t u)"))
            # 4 PE transposes into one PSUM bank, one strided copy out
            trp = pools["trpp"].tile([128, ET, 128], bf16, tag="trp",
                                     name=f"trp{sblk}")
            for et in range(ET):
                nc.tensor.transpose(trp[:, et, :],
                                    qo[:, et * 128:(et + 1) * 128], ident)
            nc.vector.tensor_copy(
                dstT[:, :, sblk * 128:(sblk + 1) * 128], trp)

        def proj_wave(w_sb, dstT, st, su):
            """project one su-128 block of si-512 block st for one tensor."""
            ps = pools["util"].tile([128, E], f32, tag="u512",
                                    name=f"pp{st}_{su}")
            for dt in range(DT):
                nc.tensor.matmul(
                    ps,
                    xfull[:, dt, st * 512 + su * 128:
                          st * 512 + (su + 1) * 128],
                    w_sb[:, dt, :],
                    start=(dt == 0), stop=(dt == DT - 1))
            if dstT is None:
                nc.vector.tensor_copy(
                    v_sb[:, st * 4 + su, :, 0:64],
                    ps.rearrange("p (h d) -> p h d", d=DH))
            else:
                proj_chain(ps, dstT, st, su)

        def attn_pair(et, i):
            """head pair (2*et, 2*et+1), si-512 block i: QK/exp/mask/PV and
            softmax division; writes attn output into qTall block i."""
            hA, hB = 2 * et, 2 * et + 1
            nsj = 4 * i + 4
            pvA = pools["pvp"].tile([128, 512], f32, tag="pv",
                                    name=f"pvA{et}_{i}")
            pvB = pools["pvp"].tile([128, 512], f32, tag="pv",
                                    name=f"pvB{et}_{i}")
            exs = {}

            def emit_qk(sjb):
                lg = pools["lgp"].tile([128, 2, 512], f32, tag="lg",
                                       name=f"lg{et}_{i}_{sjb}")
                c0 = sjb * 128
                nc.tensor.matmul(
                    lg[:, 0, :],
                    kTall[0:64, et, c0:c0 + 128],
                    qTall[0:64, et, i * 512:(i + 1) * 512],
                    start=True, stop=True)
                nc.tensor.matmul(
                    lg[:, 1, :],
                    kTall[64:128, et, c0:c0 + 128],
                    qTall[64:128, et, i * 512:(i + 1) * 512],
                    start=True, stop=True)
                ex = attnp.tile([128, 2, 512], bf16, tag="ex", bufs=3)
                nc.scalar.activation(ex, lg, AF.Exp)
                r = sjb - 4 * i
                if r >= 0:
                    nc.vector.tensor_mul(
                        ex, ex,
                        mask_sb[:, r:r + 1, :].broadcast_to([128, 2, 512]))
                exs[sjb] = ex

            emit_qk(0)
            for sjb in range(nsj):
                if sjb + 1 < nsj:
                    emit_qk(sjb + 1)
                ex = exs.pop(sjb)
                nc.tensor.matmul(
                    pvA[0:65, :], v_sb[:, sjb, hA, :], ex[:, 0, :],
                    start=(sjb == 0), stop=(sjb == nsj - 1))
                nc.tensor.matmul(
                    pvB[0:65, :], v_sb[:, sjb, hB, :], ex[:, 1, :],
                    start=(sjb == 0), stop=(sjb == nsj - 1))

            # softmax denominators -> broadcast 1/den -> scale outputs
            nc.vector.tensor_copy(den33[0:1, :], pvA[64:65, :])
            nc.vector.tensor_copy(den33[32:33, :], pvB[64:65, :])
            # in-place 1/x over rows 0..32 (rows 1-31 unused; base-partition-0
            # form because custom-DVE dst at base 32 miswrites on HW)
            nc.vector.reciprocal_approx_fast(den33, den33)
            nc.vector.tensor_copy(den33b[0:1, :], den33[0:1, :])
            nc.vector.tensor_copy(den33b[32:33, :], den33[32:33, :])
            bc = pools["lgp"].tile([128, 2, 512], f32, tag="lg",
                                   name=f"bc{et}_{i}")
            nc.tensor.matmul(bc[:, 0, :], onemap, den33b, start=True,
                             stop=True)
            bcs = attnp.tile([128, 512], bf16, tag="bcs", bufs=2)
            nc.vector.tensor_copy(bcs, bc[:, 0, :])
            nc.vector.tensor_mul(
                qTall[0:64, et, i * 512:(i + 1) * 512],
                pvA[0:64, :], bcs[0:64, :])
            nc.vector.tensor_mul(
                qTall[64:128, et, i * 512:(i + 1) * 512],
                pvB[0:64, :], bcs[64:128, :])

        def yproj_group(ib, nd):
            """one nd-512 chunk of si-128 block ib -> Y[ib*128:+128, nd*512:]."""
            ps = pools["util2"].tile([128, 512], f32, tag="y512",
                                     name=f"yp{ib}_{nd}")
            for ket in range(ET):
                nc.tensor.matmul(
                    ps,
                    qTall[:, ket, ib * 128:(ib + 1) * 128],
                    wo_sb[:, ket, nd * 512:(nd + 1) * 512],
                    start=(ket == 0), stop=(ket == ET - 1))
            ys = ypool.tile([128, 512], bf16, tag="y")
            nc.vector.tensor_copy(ys, ps)
            nc.sync.dma_start(
                Y[ib * 128:(ib + 1) * 128, nd * 512:(nd + 1) * 512], ys)

        # proj phase (own PSUM pools: 4 wave banks + 4 transpose banks)
        with tc.tile_pool(name="util", bufs=4, space="PSUM") as p_util, \
             tc.tile_pool(name="trpp", bufs=4, space="PSUM") as p_trpp:
            pools["util"] = p_util
            pools["trpp"] = p_trpp
            for st in range(SB):
                for su in range(4):
                    proj_wave(wq_sb, qTall, st, su)
                for su in range(4):
                    proj_wave(wk_sb, kTall, st, su)
                for su in range(4):
                    proj_wave(wv_sb, None, st, su)
        # attn phase: lg 2x2 banks + pv 3 + yproj/bc 1
        with tc.tile_pool(name="lgp", bufs=2, space="PSUM") as p_lgp, \
             tc.tile_pool(name="pvp", bufs=2, space="PSUM") as p_pvp, \
             tc.tile_pool(name="util2", bufs=2, space="PSUM") as p_util2:
            pools["lgp"] = p_lgp
            pools["pvp"] = p_pvp
            pools["util2"] = p_util2
            for i in range(SB):
                for et in range(ET):
                    attn_pair(et, i)
                    if i >= 1:
                        for nd in range(4):
                            yproj_group(4 * (i - 1) + et, nd)
            for ib in range(12, 16):
                for nd in range(4):
                    yproj_group(ib, nd)
    return nc


def _host_prep(x, wq, wk, wv, wo, qk_scale):
    """Returns per-core input dicts."""
    perm = np.concatenate([np.arange(0, DH, 2), np.arange(1, DH, 2)])
    wq_n = _l2n(wq, -1).reshape(HEADS, DH, DIM)[:, perm, :].reshape(HEADS * DH, DIM)
    wk_n = _l2n(wk, -1).reshape(HEADS, DH, DIM)[:, perm, :].reshape(HEADS * DH, DIM)
    wv_n = _l2n(wv, -1)
    wo_n = _l2n(wo, 0)
    sp = qk_scale.astype(np.float64)[perm]

    # rope tables with qk_scale folded in; permuted-block layout
    half = np.arange(0, DH, 2)
    freqs = 1.0 / (THETA ** (half.astype(np.float64) / DH))      # (32,)
    ang = np.arange(S, dtype=np.float64)[:, None] * freqs[None]  # (S, 32)
    cos_h, sin_h = np.cos(ang), np.sin(ang)
    cos_p = np.concatenate([cos_h, cos_h], 1)                    # (S, 64)
    sin_e = np.concatenate([-sin_h, sin_h], 1)
    cos_eff = (cos_p * sp[None, :]).astype(np.float32)
    swap_sp = np.concatenate([sp[32:], sp[:32]])
    sin_eff = (sin_e * swap_sp[None, :]).astype(np.float32)
    # device layout [128, SS*DH]: [p, b*64+c] = tbl[b*128+p, c]
    cosd = np.ascontiguousarray(
        cos_eff.reshape(SS, 128, DH).transpose(1, 0, 2).reshape(128, SS * DH))
    sind = np.ascontiguousarray(
        sin_eff.reshape(SS, 128, DH).transpose(1, 0, 2).reshape(128, SS * DH))

    # causal masks for the 4 diagonal offsets: keep sjl + 128r <= sil
    sjl = np.arange(128)[:, None]
    sil = np.arange(512)[None, :]
    maskd = np.ascontiguousarray(np.concatenate(
        [(sjl + 128 * r <= sil).astype(np.float32) for r in range(4)],
        axis=1))  # [128, 4*512]

    in_maps = []
    for c in range(NCORES):
        b, t = divmod(c, TP)
        e0 = t * E
        in_maps.append({
            "xT": np.ascontiguousarray(x[b].T).astype(BF16),
            "wqT": np.ascontiguousarray(wq_n[e0:e0 + E].T).astype(BF16),
            "wkT": np.ascontiguousarray(wk_n[e0:e0 + E].T).astype(BF16),
            "wvT": np.ascontiguousarray(wv_n[e0:e0 + E].T).astype(BF16),
            "woT": np.ascontiguousarray(wo_n[:, e0:e0 + E].T).astype(BF16),
            "cosd": cosd.astype(BF16), "sind": sind.astype(BF16),
            "maskd": maskd.astype(BF16),
        })
    return in_maps


def _install_profile_hook():
    """antenv.axon_hooks is absent in this image; shim it and register the
    ctypes NTFF hook against /opt/axon/libaxon_pjrt.so (mirrors trn_boot)."""
    import types
    import ctypes
    import contextlib

    try:
        from antenv.axon_hooks import get_axon_ntff_profile_hook  # noqa
        return
    except ImportError:
        pass
    import antenv
    mod = types.ModuleType("antenv.axon_hooks")
    state = {}
    mod.set_axon_ntff_profile_hook = lambda h: state.__setitem__("h", h)
    mod.get_axon_ntff_profile_hook = lambda: state.get("h")
    sys.modules["antenv.axon_hooks"] = mod
    antenv.axon_hooks = mod

    so_path = "/opt/axon/libaxon_pjrt.so"
    lib = ctypes.CDLL(so_path)
    if not hasattr(lib, "axon_start_nrt_profile"):
        return
    lib.axon_start_nrt_profile.argtypes = [
        ctypes.POINTER(ctypes.c_int64), ctypes.c_size_t]
    lib.axon_start_nrt_profile.restype = ctypes.c_int64
    lib.axon_stop_nrt_profile.argtypes = [ctypes.c_char_p]
    lib.axon_stop_nrt_profile.restype = ctypes.c_int64

    @contextlib.contextmanager
    def _hook(output_dir, device_ids):
        import jax
        jax.devices()
        if device_ids:
            ids = (ctypes.c_int64 * len(device_ids))(*device_ids)
            rc = lib.axon_start_nrt_profile(ids, len(device_ids))
        else:
            rc = lib.axon_start_nrt_profile(None, 0)
        if rc != 0:
            raise RuntimeError(f"axon_start_nrt_profile rc={rc}")
        try:
            yield
        finally:
            n = lib.axon_stop_nrt_profile(str(output_dir).encode())
            print(f"profile: {n} file(s) written to {output_dir}",
                  file=sys.stderr)

    mod.set_axon_ntff_profile_hook(_hook)


def kernel(x, wq, wk, wv, wo, qk_scale, _profile=False):
    from concourse.bass_utils import run_bass_kernel_spmd

    if _profile:
        _install_profile_hook()

    if "nc" not in _CACHE:
        nc = _build_program()
        nc.finalize()
        _CACHE["nc"] = nc
    nc = _CACHE["nc"]
    in_maps = _host_prep(np.asarray(x), np.asarray(wq), np.asarray(wk),
                         np.asarray(wv), np.asarray(wo), np.asarray(qk_scale))
    res = run_bass_kernel_spmd(nc, in_maps, core_ids=list(range(NCORES)),
                               trace=_profile)
    outs = res.results
    y = np.empty((B, S, DIM), dtype=np.float32)
    for b in range(B):
        y[b] = sum(np.asarray(outs[b * TP + t]["Y"], dtype=np.float32)
                   for t in range(TP))
    if _profile:
        _CACHE["last_exec_time_ns"] = res.exec_time_ns
        _CACHE["last_profile"] = res.profile_json
    return y
